# revision 8
# baseline (speedup 1.0000x reference)
"""AdaptiveSSM forward on 8 Trainium2 NeuronCores (data-parallel over N).

y_t, u_t = SSM(h_t, u_prev, W_delta, b_delta, log_A, B, C)
  delta = softplus(h @ W^T + b)                  (N, H)
  u_t   = u_prev * exp(delta[:,:,None]*A) + delta[:,:,None]*h[:,:,None]*B
  y_t   = sum_s(u_t * C)

Per-core layout: rows on SBUF partitions (tiles of 128 rows), (H,S)=2048 on
the free dim (contiguous per row -> full-rate DMA).  The broadcasts
delta (x) A and (delta*h) (x) B are produced by the TensorEngine as matmuls
against host-precomputed block-diagonal constants MA/MB
(MA[k, h*16+s] = A[h,s] if k==h else 0) with the transposed delta as the
stationary operand, so they land in PSUM without any DVE broadcast pass.
delta itself is computed transposed ([H, n]) so the Linear bias and the
softplus run on the Scalar engine with a per-partition bias.
"""

import sys
from contextlib import ExitStack

sys.path.insert(0, "/opt/trn_rl_repo")

import numpy as np

import concourse.bass as bass
import concourse.tile as tile
from concourse import bacc, mybir
from concourse.bass_utils import run_bass_kernel_spmd

N = 50000
HID = 128
STATE = 16
F = HID * STATE  # 2048
NCORES = 8
R = N // NCORES  # 6250 rows per core
TILE_ROWS = 128
CHUNK = 1024  # free-dim chunk for the elementwise pipeline (2 PSUM banks)
MM_N = 512  # max fp32 moving free dim per matmul
NEWTON_ITERS = 3

F32 = mybir.dt.float32
BF16 = mybir.dt.bfloat16


def _row_tiles():
    tiles = []
    r0 = 0
    while r0 < R:
        nt = min(TILE_ROWS, R - r0)
        tiles.append((r0, nt))
        r0 += nt
    return tiles


def build():
    nc = bacc.Bacc()

    h_d = nc.declare_dram_parameter("h_t", [R, HID], F32, isOutput=False)
    u_d = nc.declare_dram_parameter("u_prev", [R, F], F32, isOutput=False)
    wdT_d = nc.declare_dram_parameter("WdT", [HID, HID], F32, isOutput=False)
    bD_d = nc.declare_dram_parameter("bD", [HID, 1], F32, isOutput=False)
    MAh_d = nc.declare_dram_parameter("MAh", [HID, F], BF16, isOutput=False)
    MAl_d = nc.declare_dram_parameter("MAl", [HID, F], BF16, isOutput=False)
    MBh_d = nc.declare_dram_parameter("MBh", [HID, F], BF16, isOutput=False)
    Crep_d = nc.declare_dram_parameter("Crep", [HID, F], F32, isOutput=False)
    ident_d = nc.declare_dram_parameter("ident", [HID, HID], F32, isOutput=False)
    nwc_d = nc.declare_dram_parameter("nwc", [HID, NEWTON_ITERS + 1], F32, isOutput=False)
    y_d = nc.declare_dram_parameter("y_t", [R, HID], F32, isOutput=True)
    ut_d = nc.declare_dram_parameter("u_t", [R, F], F32, isOutput=True)

    with tile.TileContext(nc) as tc:
        with ExitStack() as ctx:
            ep = ctx.enter_context
            consts = ep(tc.tile_pool(name="consts", bufs=1))
            p_u = ep(tc.tile_pool(name="u_in", bufs=3))
            p_h = ep(tc.tile_pool(name="h_in", bufs=2))
            p_hT = ep(tc.tile_pool(name="hT", bufs=2))
            p_dT = ep(tc.tile_pool(name="deltaT", bufs=2))
            p_gT = ep(tc.tile_pool(name="gT", bufs=2))
            p_z0 = ep(tc.tile_pool(name="z0", bufs=2))
            p_z = ep(tc.tile_pool(name="z", bufs=2))
            p_q = ep(tc.tile_pool(name="q", bufs=2))
            p_w = ep(tc.tile_pool(name="w", bufs=2))
            p_p = ep(tc.tile_pool(name="p", bufs=2))
            p_dhi = ep(tc.tile_pool(name="dhi", bufs=2))
            p_dlo = ep(tc.tile_pool(name="dlo", bufs=2))
            p_ghi = ep(tc.tile_pool(name="ghi", bufs=2))
            p_e = ep(tc.tile_pool(name="e", bufs=3))
            p_t2 = ep(tc.tile_pool(name="t2", bufs=3))
            p_yt = ep(tc.tile_pool(name="yt", bufs=3))
            p_ut = ep(tc.tile_pool(name="ut", bufs=3))
            p_y = ep(tc.tile_pool(name="y", bufs=3))
            p_psT = ep(tc.tile_pool(name="psT", bufs=2, space="PSUM"))
            p_psD = ep(tc.tile_pool(name="psD", bufs=2, space="PSUM"))
            p_psA = ep(tc.tile_pool(name="psA", bufs=1, space="PSUM"))
            p_psB = ep(tc.tile_pool(name="psB", bufs=1, space="PSUM"))

            wdT = consts.tile([HID, HID], F32)
            nc.sync.dma_start(out=wdT[:], in_=wdT_d[:, :])
            bD = consts.tile([HID, 1], F32)
            nc.sync.dma_start(out=bD[:], in_=bD_d[:, :])
            MAh = consts.tile([HID, F], BF16)
            nc.sync.dma_start(out=MAh[:], in_=MAh_d[:, :])
            MAl = consts.tile([HID, F], BF16)
            nc.sync.dma_start(out=MAl[:], in_=MAl_d[:, :])
            MBh = consts.tile([HID, F], BF16)
            nc.sync.dma_start(out=MBh[:], in_=MBh_d[:, :])
            Crep = consts.tile([HID, F], F32)
            nc.sync.dma_start(out=Crep[:], in_=Crep_d[:, :])
            ident = consts.tile([HID, HID], F32)
            nc.sync.dma_start(out=ident[:], in_=ident_d[:, :])
            nwc = consts.tile([HID, NEWTON_ITERS + 1], F32)
            nc.sync.dma_start(out=nwc[:], in_=nwc_d[:, :])

            for r0, nt in _row_tiles():
                u_in = p_u.tile([nt, F], F32)
                nc.sync.dma_start(out=u_in[:], in_=u_d[r0:r0 + nt, :])
                h_in = p_h.tile([nt, HID], F32)
                nc.sync.dma_start(out=h_in[:], in_=h_d[r0:r0 + nt, :])

                # hT[h_in, n] = h^T via PE transpose
                psT = p_psT.tile([HID, nt], F32)
                nc.tensor.transpose(psT[:], h_in[:], ident[:nt, :nt])
                hT = p_hT.tile([HID, nt], F32)
                nc.scalar.copy(hT[:], psT[:])

                # delta_preT[h_out, n] = W @ h^T  (full fp32 for accuracy)
                psD = p_psD.tile([HID, nt], F32)
                nc.tensor.matmul(psD[:], lhsT=wdT[:], rhs=hT[:], start=True, stop=True)

                # deltaT = softplus(pre + b) = ln(z), z = 1 + e^(pre+b).
                # No Softplus/Ln ACT table on this stack -> Newton from
                # d0 = relu(x):  d_{k+1} = d_k - 1 + z*exp(-d_k).
                # Track q_k = d_k + k so the -1 folds into the exp bias.
                z0 = p_z0.tile([HID, nt], F32)
                nc.scalar.activation(
                    z0[:], psD[:], mybir.ActivationFunctionType.Exp, bias=bD[:],
                )
                z = p_z.tile([HID, nt], F32)
                nc.vector.tensor_scalar_add(z[:], z0[:], 1.0)
                q = p_q.tile([HID, nt], F32)
                nc.scalar.activation(
                    q[:], psD[:], mybir.ActivationFunctionType.Relu, bias=bD[:],
                )
                for k in range(NEWTON_ITERS):
                    w = p_w.tile([HID, nt], F32)
                    nc.scalar.activation(
                        w[:], q[:], mybir.ActivationFunctionType.Exp,
                        scale=-1.0, bias=nwc[:, k:k + 1],
                    )
                    p = p_p.tile([HID, nt], F32)
                    nc.gpsimd.tensor_tensor(p[:], z[:], w[:], mybir.AluOpType.mult)
                    qn = p_q.tile([HID, nt], F32)
                    nc.gpsimd.tensor_tensor(qn[:], q[:], p[:], mybir.AluOpType.add)
                    q = qn
                deltaT = p_dT.tile([HID, nt], F32)
                nc.scalar.activation(
                    deltaT[:], q[:], mybir.ActivationFunctionType.Identity,
                    bias=nwc[:, NEWTON_ITERS:NEWTON_ITERS + 1],
                )
                gT = p_gT.tile([HID, nt], F32)
                nc.vector.tensor_mul(gT[:], deltaT[:], hT[:])
                # bf16 hi/lo split of deltaT for the exact-ish dA matmul;
                # single bf16 for the B term (its error is not exp-amplified).
                d_hi = p_dhi.tile([HID, nt], BF16)
                nc.scalar.copy(d_hi[:], deltaT[:])
                d_lo = p_dlo.tile([HID, nt], BF16)
                nc.gpsimd.tensor_tensor(
                    d_lo[:], deltaT[:], d_hi[:], mybir.AluOpType.subtract
                )
                g_hi = p_ghi.tile([HID, nt], BF16)
                nc.scalar.copy(g_hi[:], gT[:])

                ut = p_ut.tile([nt, F], F32)
                y = p_y.tile([nt, HID], F32)

                for c in range(F // CHUNK):
                    f0 = c * CHUNK
                    # dA[n, (h s)] = deltaT^T @ MA  (fp32r full-rate)
                    psA = p_psA.tile([nt, CHUNK], F32)
                    psB = p_psB.tile([nt, CHUNK], F32)
                    for j in range(CHUNK // MM_N):
                        m0 = j * MM_N
                        sl = slice(f0 + m0, f0 + m0 + MM_N)
                        nc.tensor.matmul(
                            psA[:, m0:m0 + MM_N], lhsT=d_hi[:],
                            rhs=MAh[:, sl], start=True, stop=False,
                        )
                        nc.tensor.matmul(
                            psA[:, m0:m0 + MM_N], lhsT=d_lo[:],
                            rhs=MAh[:, sl], start=False, stop=False,
                        )
                        nc.tensor.matmul(
                            psA[:, m0:m0 + MM_N], lhsT=d_hi[:],
                            rhs=MAl[:, sl], start=False, stop=True,
                        )
                        nc.tensor.matmul(
                            psB[:, m0:m0 + MM_N], lhsT=g_hi[:],
                            rhs=MBh[:, sl], start=True, stop=True,
                        )
                    e = p_e.tile([nt, CHUNK], F32)
                    nc.scalar.activation(
                        e[:], psA[:], mybir.ActivationFunctionType.Exp,
                    )
                    t2 = p_t2.tile([nt, CHUNK], F32)
                    nc.vector.tensor_mul(t2[:], u_in[:, f0:f0 + CHUNK], e[:])
                    nc.vector.tensor_add(ut[:, f0:f0 + CHUNK], t2[:], psB[:])
                    yt = p_yt.tile([nt, CHUNK], F32)
                    nc.gpsimd.tensor_tensor(
                        yt[:], ut[:, f0:f0 + CHUNK], Crep[:nt, f0:f0 + CHUNK],
                        mybir.AluOpType.mult,
                    )
                    h0 = f0 // STATE
                    nh = CHUNK // STATE
                    nc.vector.tensor_reduce(
                        y[:, h0:h0 + nh],
                        yt[:].rearrange("p (h s) -> p h s", s=STATE),
                        axis=mybir.AxisListType.X,
                        op=mybir.AluOpType.add,
                    )

                nc.sync.dma_start(out=ut_d[r0:r0 + nt, :], in_=ut[:])
                nc.sync.dma_start(out=y_d[r0:r0 + nt, :], in_=y[:])

    nc.compile()
    return nc


_NC = None


def _get_nc():
    global _NC
    if _NC is None:
        _NC = build()
    return _NC


def _host_consts(W_delta, b_delta, log_A, B, C):
    import ml_dtypes

    A = -np.exp(log_A.astype(np.float64)).astype(np.float32)  # (H, S)
    idx = np.arange(HID)
    MA = np.zeros((HID, F), dtype=np.float32)
    MA[idx[:, None], idx[:, None] * STATE + np.arange(STATE)[None, :]] = A
    MB = np.zeros((HID, F), dtype=np.float32)
    MB[idx[:, None], idx[:, None] * STATE + np.arange(STATE)[None, :]] = B.astype(
        np.float32
    )
    MAh = MA.astype(ml_dtypes.bfloat16)
    MAl = (MA - MAh.astype(np.float32)).astype(ml_dtypes.bfloat16)
    MBh = MB.astype(ml_dtypes.bfloat16)
    Crep = np.broadcast_to(
        C.astype(np.float32).reshape(1, F), (HID, F)
    ).copy()
    return {
        "WdT": np.ascontiguousarray(W_delta.T.astype(np.float32)),
        "bD": b_delta.astype(np.float32).reshape(HID, 1),
        "MAh": MAh,
        "MAl": MAl,
        "MBh": MBh,
        "Crep": Crep,
        "ident": np.eye(HID, dtype=np.float32),
        "nwc": np.broadcast_to(
            np.array(
                list(range(NEWTON_ITERS)) + [-float(NEWTON_ITERS)], dtype=np.float32
            ),
            (HID, NEWTON_ITERS + 1),
        ).copy(),
    }


def kernel(h_t, u_prev, W_delta, b_delta, log_A, B, C):
    h_t = np.asarray(h_t, dtype=np.float32)
    u_prev = np.asarray(u_prev, dtype=np.float32)
    consts = _host_consts(
        np.asarray(W_delta), np.asarray(b_delta), np.asarray(log_A),
        np.asarray(B), np.asarray(C),
    )
    u_flat = np.ascontiguousarray(u_prev.reshape(N, F))
    in_maps = []
    for i in range(NCORES):
        r0 = i * R
        m = {
            "h_t": np.ascontiguousarray(h_t[r0:r0 + R]),
            "u_prev": u_flat[r0:r0 + R],
        }
        m.update(consts)
        in_maps.append(m)
    nc = _get_nc()
    res = run_bass_kernel_spmd(nc, in_maps, list(range(NCORES)))
    y = np.concatenate([res.results[i]["y_t"] for i in range(NCORES)], axis=0)
    ut = np.concatenate([res.results[i]["u_t"] for i in range(NCORES)], axis=0)
    return y, ut.reshape(N, HID, STATE)


# revision 9
# speedup vs baseline: 1.1218x; 1.1218x over previous
"""AdaptiveSSM forward on 8 Trainium2 NeuronCores (data-parallel over N).

y_t, u_t = SSM(h_t, u_prev, W_delta, b_delta, log_A, B, C)
  delta = softplus(h @ W^T + b)                  (N, H)
  u_t   = u_prev * exp(delta[:,:,None]*A) + delta[:,:,None]*h[:,:,None]*B
  y_t   = sum_s(u_t * C)

Per-core layout: rows on SBUF partitions (tiles of 128 rows), (H,S)=2048 on
the free dim (contiguous per row -> full-rate DMA).  The broadcasts
delta (x) A and (delta*h) (x) B are produced by the TensorEngine as matmuls
against host-precomputed block-diagonal constants MA/MB
(MA[k, h*16+s] = A[h,s] if k==h else 0) with the transposed delta as the
stationary operand, so they land in PSUM without any DVE broadcast pass.
delta itself is computed transposed ([H, n]) so the Linear bias and the
softplus run on the Scalar engine with a per-partition bias.
"""

import sys
from contextlib import ExitStack

sys.path.insert(0, "/opt/trn_rl_repo")

import numpy as np

import concourse.bass as bass
import concourse.tile as tile
from concourse import bacc, mybir
from concourse.bass_utils import run_bass_kernel_spmd

N = 50000
HID = 128
STATE = 16
F = HID * STATE  # 2048
NCORES = 8
R = N // NCORES  # 6250 rows per core
TILE_ROWS = 128
CHUNK = 1024  # free-dim chunk for the elementwise pipeline (2 PSUM banks)
MM_N = 512  # max fp32 moving free dim per matmul
NEWTON_ITERS = 3
GROUP_ROWS = 512  # rows per delta-phase group (4 row-tiles)

F32 = mybir.dt.float32
BF16 = mybir.dt.bfloat16


def _row_tiles():
    tiles = []
    r0 = 0
    while r0 < R:
        nt = min(TILE_ROWS, R - r0)
        tiles.append((r0, nt))
        r0 += nt
    return tiles


def _groups():
    """Partition R rows into groups of row-tiles: [(g0, [(r0, nt), ...]), ...]"""
    out = []
    g0 = 0
    while g0 < R:
        gr = min(GROUP_ROWS, R - g0)
        tiles = []
        r0 = g0
        while r0 < g0 + gr:
            nt = min(TILE_ROWS, g0 + gr - r0)
            tiles.append((r0, nt))
            r0 += nt
        out.append((g0, gr, tiles))
        g0 += gr
    return out


def build():
    nc = bacc.Bacc()

    h_d = nc.declare_dram_parameter("h_t", [R, HID], F32, isOutput=False)
    u_d = nc.declare_dram_parameter("u_prev", [R, F], F32, isOutput=False)
    wdT_d = nc.declare_dram_parameter("WdT", [HID, HID], F32, isOutput=False)
    bD_d = nc.declare_dram_parameter("bD", [HID, 1], F32, isOutput=False)
    MAh_d = nc.declare_dram_parameter("MAh", [HID, F], BF16, isOutput=False)
    MAl_d = nc.declare_dram_parameter("MAl", [HID, F], BF16, isOutput=False)
    MBh_d = nc.declare_dram_parameter("MBh", [HID, F], BF16, isOutput=False)
    Crep_d = nc.declare_dram_parameter("Crep", [HID, F], F32, isOutput=False)
    ident_d = nc.declare_dram_parameter("ident", [HID, HID], F32, isOutput=False)
    y_d = nc.declare_dram_parameter("y_t", [R, HID], F32, isOutput=True)
    ut_d = nc.declare_dram_parameter("u_t", [R, F], F32, isOutput=True)

    Exp = mybir.ActivationFunctionType.Exp
    Relu = mybir.ActivationFunctionType.Relu

    with tile.TileContext(nc) as tc:
        with ExitStack() as ctx:
            ep = ctx.enter_context
            consts = ep(tc.tile_pool(name="consts", bufs=1))
            p_u = ep(tc.tile_pool(name="u_in", bufs=3))
            p_h = ep(tc.tile_pool(name="h_in", bufs=3))
            p_hTg = ep(tc.tile_pool(name="hTg", bufs=2))
            p_z0 = ep(tc.tile_pool(name="z0", bufs=2))
            p_z = ep(tc.tile_pool(name="z", bufs=2))
            p_q0 = ep(tc.tile_pool(name="q0", bufs=2))
            p_w = ep(tc.tile_pool(name="w", bufs=2))
            p_p1 = ep(tc.tile_pool(name="p1", bufs=2))
            p_p2 = ep(tc.tile_pool(name="p2", bufs=2))
            p_p3 = ep(tc.tile_pool(name="p3", bufs=2))
            p_a = ep(tc.tile_pool(name="a", bufs=2))
            p_dT = ep(tc.tile_pool(name="dT", bufs=2))
            p_gT = ep(tc.tile_pool(name="gT", bufs=2))
            p_dhi = ep(tc.tile_pool(name="dhi", bufs=2))
            p_dlo = ep(tc.tile_pool(name="dlo", bufs=2))
            p_ghi = ep(tc.tile_pool(name="ghi", bufs=2))
            p_e = ep(tc.tile_pool(name="e", bufs=3))
            p_t2 = ep(tc.tile_pool(name="t2", bufs=3))
            p_yt = ep(tc.tile_pool(name="yt", bufs=3))
            p_ut = ep(tc.tile_pool(name="ut", bufs=3))
            p_y = ep(tc.tile_pool(name="y", bufs=3))
            p_psT = ep(tc.tile_pool(name="psT", bufs=1, space="PSUM"))
            p_psD = ep(tc.tile_pool(name="psD", bufs=1, space="PSUM"))
            p_psA = ep(tc.tile_pool(name="psA", bufs=2, space="PSUM"))
            p_psB = ep(tc.tile_pool(name="psB", bufs=1, space="PSUM"))

            wdT = consts.tile([HID, HID], F32)
            nc.sync.dma_start(out=wdT[:], in_=wdT_d[:, :])
            bD = consts.tile([HID, 1], F32)
            nc.sync.dma_start(out=bD[:], in_=bD_d[:, :])
            MAh = consts.tile([HID, F], BF16)
            nc.sync.dma_start(out=MAh[:], in_=MAh_d[:, :])
            MAl = consts.tile([HID, F], BF16)
            nc.sync.dma_start(out=MAl[:], in_=MAl_d[:, :])
            MBh = consts.tile([HID, F], BF16)
            nc.sync.dma_start(out=MBh[:], in_=MBh_d[:, :])
            Crep = consts.tile([HID, F], F32)
            nc.sync.dma_start(out=Crep[:], in_=Crep_d[:, :])
            ident = consts.tile([HID, HID], F32)
            nc.sync.dma_start(out=ident[:], in_=ident_d[:, :])

            def phase_a(g0, gr, tiles):
                """delta/softplus for one group -> (d_hi, d_lo, g_hi) bf16."""
                hTg = p_hTg.tile([HID, gr], F32)
                for r0, nt in tiles:
                    h_in = p_h.tile([nt, HID], F32)
                    nc.sync.dma_start(out=h_in[:], in_=h_d[r0:r0 + nt, :])
                    psT = p_psT.tile([HID, nt], F32)
                    nc.tensor.transpose(psT[:], h_in[:], ident[:nt, :nt])
                    nc.scalar.copy(hTg[:, r0 - g0:r0 - g0 + nt], psT[:])
                psD = p_psD.tile([HID, gr], F32)
                nc.tensor.matmul(psD[:], lhsT=wdT[:], rhs=hTg[:], start=True, stop=True)
                # softplus(pre+b) via Newton: d_K = relu + p1+..+pK - K,
                # p1 = (1+e^x)e^{-relu(x)}, p_{k+1} = p_k * exp(1 - p_k)
                z0 = p_z0.tile([HID, gr], F32)
                nc.scalar.activation(z0[:], psD[:], Exp, bias=bD[:])
                z = p_z.tile([HID, gr], F32)
                nc.vector.tensor_scalar_add(z[:], z0[:], 1.0)
                q0 = p_q0.tile([HID, gr], F32)
                nc.scalar.activation(q0[:], psD[:], Relu, bias=bD[:])
                w = p_w.tile([HID, gr], F32)
                nc.scalar.activation(w[:], q0[:], Exp, scale=-1.0)
                p1 = p_p1.tile([HID, gr], F32)
                nc.gpsimd.tensor_tensor(p1[:], z[:], w[:], mybir.AluOpType.mult)
                w2 = p_w.tile([HID, gr], F32)
                nc.scalar.activation(w2[:], p1[:], Exp, scale=-1.0, bias=1.0)
                p2 = p_p2.tile([HID, gr], F32)
                nc.gpsimd.tensor_tensor(p2[:], p1[:], w2[:], mybir.AluOpType.mult)
                w3 = p_w.tile([HID, gr], F32)
                nc.scalar.activation(w3[:], p2[:], Exp, scale=-1.0, bias=1.0)
                p3 = p_p3.tile([HID, gr], F32)
                nc.gpsimd.tensor_tensor(p3[:], p2[:], w3[:], mybir.AluOpType.mult)
                a1 = p_a.tile([HID, gr], F32)
                nc.vector.tensor_add(a1[:], p1[:], p2[:])
                a2 = p_a.tile([HID, gr], F32)
                nc.vector.tensor_add(a2[:], p3[:], q0[:])
                dT = p_dT.tile([HID, gr], F32)
                nc.vector.scalar_tensor_tensor(
                    dT[:], a1[:], float(-NEWTON_ITERS), a2[:],
                    mybir.AluOpType.add, mybir.AluOpType.add,
                )
                gT = p_gT.tile([HID, gr], F32)
                nc.vector.tensor_mul(gT[:], dT[:], hTg[:])
                d_hi = p_dhi.tile([HID, gr], BF16)
                nc.scalar.copy(d_hi[:], dT[:])
                d_lo = p_dlo.tile([HID, gr], BF16)
                nc.gpsimd.tensor_tensor(
                    d_lo[:], dT[:], d_hi[:], mybir.AluOpType.subtract
                )
                g_hi = p_ghi.tile([HID, gr], BF16)
                nc.scalar.copy(g_hi[:], gT[:])
                return d_hi, d_lo, g_hi

            def phase_b(g0, gr, tiles, dhdlgh):
                d_hi, d_lo, g_hi = dhdlgh
                for r0, nt in tiles:
                    j0 = r0 - g0
                    u_in = p_u.tile([nt, F], F32)
                    nc.sync.dma_start(out=u_in[:], in_=u_d[r0:r0 + nt, :])
                    ut = p_ut.tile([nt, F], F32)
                    y = p_y.tile([nt, HID], F32)
                    for c in range(F // CHUNK):
                        f0 = c * CHUNK
                        psA = p_psA.tile([nt, CHUNK], F32)
                        psB = p_psB.tile([nt, CHUNK], F32)
                        for j in range(CHUNK // MM_N):
                            m0 = j * MM_N
                            sl = slice(f0 + m0, f0 + m0 + MM_N)
                            nc.tensor.matmul(
                                psA[:, m0:m0 + MM_N], lhsT=d_hi[:, j0:j0 + nt],
                                rhs=MAh[:, sl], start=True, stop=False,
                            )
                            nc.tensor.matmul(
                                psA[:, m0:m0 + MM_N], lhsT=d_lo[:, j0:j0 + nt],
                                rhs=MAh[:, sl], start=False, stop=False,
                            )
                            nc.tensor.matmul(
                                psA[:, m0:m0 + MM_N], lhsT=d_hi[:, j0:j0 + nt],
                                rhs=MAl[:, sl], start=False, stop=True,
                            )
                            nc.tensor.matmul(
                                psB[:, m0:m0 + MM_N], lhsT=g_hi[:, j0:j0 + nt],
                                rhs=MBh[:, sl], start=True, stop=True,
                            )
                        e = p_e.tile([nt, CHUNK], F32)
                        nc.scalar.activation(e[:], psA[:], Exp)
                        t2 = p_t2.tile([nt, CHUNK], F32)
                        nc.vector.tensor_mul(t2[:], u_in[:, f0:f0 + CHUNK], e[:])
                        nc.vector.tensor_add(ut[:, f0:f0 + CHUNK], t2[:], psB[:])
                        yt = p_yt.tile([nt, CHUNK], F32)
                        nc.gpsimd.tensor_tensor(
                            yt[:], ut[:, f0:f0 + CHUNK], Crep[:nt, f0:f0 + CHUNK],
                            mybir.AluOpType.mult,
                        )
                        h0 = f0 // STATE
                        nh = CHUNK // STATE
                        nc.vector.tensor_reduce(
                            y[:, h0:h0 + nh],
                            yt[:].rearrange("p (h s) -> p h s", s=STATE),
                            axis=mybir.AxisListType.X,
                            op=mybir.AluOpType.add,
                        )
                    nc.scalar.dma_start(out=ut_d[r0:r0 + nt, :], in_=ut[:])
                    nc.scalar.dma_start(out=y_d[r0:r0 + nt, :], in_=y[:])

            # software pipeline: delta phase of group i+1 is emitted before the
            # streaming phase of group i
            groups = _groups()
            pend = []
            for gi, (g0, gr, tiles) in enumerate(groups):
                pend.append((g0, gr, tiles, phase_a(g0, gr, tiles)))
                if gi >= 1:
                    b = pend.pop(0)
                    phase_b(b[0], b[1], b[2], b[3])
            for b in pend:
                phase_b(b[0], b[1], b[2], b[3])

    nc.compile()
    return nc


_NC = None


def _get_nc():
    global _NC
    if _NC is None:
        _NC = build()
    return _NC


def _host_consts(W_delta, b_delta, log_A, B, C):
    import ml_dtypes

    A = -np.exp(log_A.astype(np.float64)).astype(np.float32)  # (H, S)
    idx = np.arange(HID)
    MA = np.zeros((HID, F), dtype=np.float32)
    MA[idx[:, None], idx[:, None] * STATE + np.arange(STATE)[None, :]] = A
    MB = np.zeros((HID, F), dtype=np.float32)
    MB[idx[:, None], idx[:, None] * STATE + np.arange(STATE)[None, :]] = B.astype(
        np.float32
    )
    MAh = MA.astype(ml_dtypes.bfloat16)
    MAl = (MA - MAh.astype(np.float32)).astype(ml_dtypes.bfloat16)
    MBh = MB.astype(ml_dtypes.bfloat16)
    Crep = np.broadcast_to(
        C.astype(np.float32).reshape(1, F), (HID, F)
    ).copy()
    return {
        "WdT": np.ascontiguousarray(W_delta.T.astype(np.float32)),
        "bD": b_delta.astype(np.float32).reshape(HID, 1),
        "MAh": MAh,
        "MAl": MAl,
        "MBh": MBh,
        "Crep": Crep,
        "ident": np.eye(HID, dtype=np.float32),
    }


def kernel(h_t, u_prev, W_delta, b_delta, log_A, B, C):
    h_t = np.asarray(h_t, dtype=np.float32)
    u_prev = np.asarray(u_prev, dtype=np.float32)
    consts = _host_consts(
        np.asarray(W_delta), np.asarray(b_delta), np.asarray(log_A),
        np.asarray(B), np.asarray(C),
    )
    u_flat = np.ascontiguousarray(u_prev.reshape(N, F))
    in_maps = []
    for i in range(NCORES):
        r0 = i * R
        m = {
            "h_t": np.ascontiguousarray(h_t[r0:r0 + R]),
            "u_prev": u_flat[r0:r0 + R],
        }
        m.update(consts)
        in_maps.append(m)
    nc = _get_nc()
    res = run_bass_kernel_spmd(nc, in_maps, list(range(NCORES)))
    y = np.concatenate([res.results[i]["y_t"] for i in range(NCORES)], axis=0)
    ut = np.concatenate([res.results[i]["u_t"] for i in range(NCORES)], axis=0)
    return y, ut.reshape(N, HID, STATE)


# revision 13
# speedup vs baseline: 1.7202x; 1.5334x over previous
"""AdaptiveSSM forward on 8 Trainium2 NeuronCores (data-parallel over N).

y_t, u_t = SSM(h_t, u_prev, W_delta, b_delta, log_A, B, C)
  delta = softplus(h @ W^T + b)                  (N, H)
  u_t   = u_prev * exp(delta[:,:,None]*A) + delta[:,:,None]*h[:,:,None]*B
  y_t   = sum_s(u_t * C)

Per-core layout: rows on SBUF partitions (tiles of 128 rows), (H,S)=2048 on
the free dim (contiguous per row -> full-rate DMA).  The broadcasts
delta (x) A and (delta*h) (x) B are produced by the TensorEngine as matmuls
against host-precomputed block-diagonal constants MA/MB
(MA[k, h*16+s] = A[h,s] if k==h else 0) with the transposed delta as the
stationary operand, so they land in PSUM without any DVE broadcast pass.
delta itself is computed transposed ([H, n]) so the Linear bias and the
softplus run on the Scalar engine with a per-partition bias.
"""

import sys
from contextlib import ExitStack

sys.path.insert(0, "/opt/trn_rl_repo")

import numpy as np

import concourse.bass as bass
import concourse.tile as tile
from concourse import bacc, mybir
from concourse.bass_utils import run_bass_kernel_spmd

N = 50000
HID = 128
STATE = 16
F = HID * STATE  # 2048
NCORES = 8
R = N // NCORES  # 6250 rows per core
TILE_ROWS = 128
CHUNK = 1024  # free-dim chunk for the elementwise pipeline (2 PSUM banks)
MM_N = 512  # max fp32 moving free dim per matmul
NEWTON_ITERS = 3
GROUP_ROWS = 512  # rows per delta-phase group (4 row-tiles)

F32 = mybir.dt.float32
BF16 = mybir.dt.bfloat16


def _row_tiles():
    tiles = []
    r0 = 0
    while r0 < R:
        nt = min(TILE_ROWS, R - r0)
        tiles.append((r0, nt))
        r0 += nt
    return tiles


def _groups():
    """Partition R rows into groups of row-tiles: [(g0, [(r0, nt), ...]), ...]"""
    out = []
    g0 = 0
    while g0 < R:
        gr = min(GROUP_ROWS, R - g0)
        tiles = []
        r0 = g0
        while r0 < g0 + gr:
            nt = min(TILE_ROWS, g0 + gr - r0)
            tiles.append((r0, nt))
            r0 += nt
        out.append((g0, gr, tiles))
        g0 += gr
    return out


def build():
    nc = bacc.Bacc()

    h_d = nc.declare_dram_parameter("h_t", [R, HID], F32, isOutput=False)
    u_d = nc.declare_dram_parameter("u_prev", [R, F], BF16, isOutput=False)
    wdT_d = nc.declare_dram_parameter("WdT", [HID, HID], F32, isOutput=False)
    bD_d = nc.declare_dram_parameter("bD", [HID, 1], F32, isOutput=False)
    MAh_d = nc.declare_dram_parameter("MAh", [HID, F], BF16, isOutput=False)
    MAl_d = nc.declare_dram_parameter("MAl", [HID, F], BF16, isOutput=False)
    MBh_d = nc.declare_dram_parameter("MBh", [HID, F], BF16, isOutput=False)
    Crep_d = nc.declare_dram_parameter("Crep", [HID, F], BF16, isOutput=False)
    ident_d = nc.declare_dram_parameter("ident", [HID, HID], F32, isOutput=False)
    y_d = nc.declare_dram_parameter("y_t", [R, HID], F32, isOutput=True)
    ut_d = nc.declare_dram_parameter("u_t", [R, F], BF16, isOutput=True)

    Exp = mybir.ActivationFunctionType.Exp
    Relu = mybir.ActivationFunctionType.Relu

    with tile.TileContext(nc) as tc:
        with ExitStack() as ctx:
            ep = ctx.enter_context
            consts = ep(tc.tile_pool(name="consts", bufs=1))
            p_u = ep(tc.tile_pool(name="u_in", bufs=3))
            p_h = ep(tc.tile_pool(name="h_in", bufs=3))
            p_hTg = ep(tc.tile_pool(name="hTg", bufs=2))
            p_z0 = ep(tc.tile_pool(name="z0", bufs=2))
            p_z = ep(tc.tile_pool(name="z", bufs=2))
            p_q0 = ep(tc.tile_pool(name="q0", bufs=2))
            p_w = ep(tc.tile_pool(name="w", bufs=2))
            p_p1 = ep(tc.tile_pool(name="p1", bufs=2))
            p_p2 = ep(tc.tile_pool(name="p2", bufs=2))
            p_p3 = ep(tc.tile_pool(name="p3", bufs=2))
            p_a = ep(tc.tile_pool(name="a", bufs=2))
            p_dT = ep(tc.tile_pool(name="dT", bufs=2))
            p_gT = ep(tc.tile_pool(name="gT", bufs=2))
            p_dhi = ep(tc.tile_pool(name="dhi", bufs=2))
            p_dlo = ep(tc.tile_pool(name="dlo", bufs=2))
            p_ghi = ep(tc.tile_pool(name="ghi", bufs=2))
            p_e = ep(tc.tile_pool(name="e", bufs=3))
            p_t2 = ep(tc.tile_pool(name="t2", bufs=3))
            p_yt = ep(tc.tile_pool(name="yt", bufs=3))
            p_t4b = ep(tc.tile_pool(name="t4b", bufs=3))
            p_ut = ep(tc.tile_pool(name="ut", bufs=3))
            p_y = ep(tc.tile_pool(name="y", bufs=3))
            p_psT = ep(tc.tile_pool(name="psT", bufs=1, space="PSUM"))
            p_psD = ep(tc.tile_pool(name="psD", bufs=1, space="PSUM"))
            p_psA = ep(tc.tile_pool(name="psA", bufs=2, space="PSUM"))
            p_psB = ep(tc.tile_pool(name="psB", bufs=1, space="PSUM"))

            wdT = consts.tile([HID, HID], F32)
            nc.sync.dma_start(out=wdT[:], in_=wdT_d[:, :])
            bD = consts.tile([HID, 1], F32)
            nc.sync.dma_start(out=bD[:], in_=bD_d[:, :])
            MAh = consts.tile([HID, F], BF16)
            nc.sync.dma_start(out=MAh[:], in_=MAh_d[:, :])
            MAl = consts.tile([HID, F], BF16)
            nc.sync.dma_start(out=MAl[:], in_=MAl_d[:, :])
            MBh = consts.tile([HID, F], BF16)
            nc.sync.dma_start(out=MBh[:], in_=MBh_d[:, :])
            Crep = consts.tile([HID, F], BF16)
            nc.sync.dma_start(out=Crep[:], in_=Crep_d[:, :])
            ident = consts.tile([HID, HID], F32)
            nc.sync.dma_start(out=ident[:], in_=ident_d[:, :])

            def phase_a(g0, gr, tiles):
                """delta/softplus for one group -> (d_hi, d_lo, g_hi) bf16."""
                hTg = p_hTg.tile([HID, gr], F32)
                for r0, nt in tiles:
                    h_in = p_h.tile([nt, HID], F32)
                    nc.sync.dma_start(out=h_in[:], in_=h_d[r0:r0 + nt, :])
                    psT = p_psT.tile([HID, nt], F32)
                    nc.tensor.transpose(psT[:], h_in[:], ident[:nt, :nt])
                    nc.scalar.copy(hTg[:, r0 - g0:r0 - g0 + nt], psT[:])
                psD = p_psD.tile([HID, gr], F32)
                nc.tensor.matmul(psD[:], lhsT=wdT[:], rhs=hTg[:], start=True, stop=True)
                # softplus(pre+b) via Newton: d_K = relu + p1+..+pK - K,
                # p1 = (1+e^x)e^{-relu(x)}, p_{k+1} = p_k * exp(1 - p_k)
                z0 = p_z0.tile([HID, gr], F32)
                nc.scalar.activation(z0[:], psD[:], Exp, bias=bD[:])
                z = p_z.tile([HID, gr], F32)
                nc.vector.tensor_scalar_add(z[:], z0[:], 1.0)
                q0 = p_q0.tile([HID, gr], F32)
                nc.scalar.activation(q0[:], psD[:], Relu, bias=bD[:])
                w = p_w.tile([HID, gr], F32)
                nc.scalar.activation(w[:], q0[:], Exp, scale=-1.0)
                p1 = p_p1.tile([HID, gr], F32)
                nc.gpsimd.tensor_tensor(p1[:], z[:], w[:], mybir.AluOpType.mult)
                w2 = p_w.tile([HID, gr], F32)
                nc.scalar.activation(w2[:], p1[:], Exp, scale=-1.0, bias=1.0)
                p2 = p_p2.tile([HID, gr], F32)
                nc.gpsimd.tensor_tensor(p2[:], p1[:], w2[:], mybir.AluOpType.mult)
                w3 = p_w.tile([HID, gr], F32)
                nc.scalar.activation(w3[:], p2[:], Exp, scale=-1.0, bias=1.0)
                p3 = p_p3.tile([HID, gr], F32)
                nc.gpsimd.tensor_tensor(p3[:], p2[:], w3[:], mybir.AluOpType.mult)
                a1 = p_a.tile([HID, gr], F32)
                nc.vector.tensor_add(a1[:], p1[:], p2[:])
                a2 = p_a.tile([HID, gr], F32)
                nc.vector.tensor_add(a2[:], p3[:], q0[:])
                dT = p_dT.tile([HID, gr], F32)
                nc.vector.scalar_tensor_tensor(
                    dT[:], a1[:], float(-NEWTON_ITERS), a2[:],
                    mybir.AluOpType.add, mybir.AluOpType.add,
                )
                gT = p_gT.tile([HID, gr], F32)
                nc.vector.tensor_mul(gT[:], dT[:], hTg[:])
                d_hi = p_dhi.tile([HID, gr], BF16)
                nc.scalar.copy(d_hi[:], dT[:])
                d_lo = p_dlo.tile([HID, gr], BF16)
                nc.gpsimd.tensor_tensor(
                    d_lo[:], dT[:], d_hi[:], mybir.AluOpType.subtract
                )
                g_hi = p_ghi.tile([HID, gr], BF16)
                nc.scalar.copy(g_hi[:], gT[:])
                return d_hi, d_lo, g_hi

            def phase_b(g0, gr, tiles, dhdlgh):
                d_hi, d_lo, g_hi = dhdlgh
                for r0, nt in tiles:
                    j0 = r0 - g0
                    u_in = p_u.tile([nt, F], BF16)
                    nc.sync.dma_start(out=u_in[:], in_=u_d[r0:r0 + nt, :])
                    ut = p_ut.tile([nt, F], BF16)
                    y = p_y.tile([nt, HID], F32)
                    psAs = []
                    psBs = []
                    nchunk = F // CHUNK
                    for c in range(nchunk):
                        psAs.append(p_psA.tile([nt, CHUNK], F32, name="psA", tag="psA"))
                        psBs.append(p_psB.tile([nt, CHUNK], F32, name="psB", tag="psB"))
                    # matmuls grouped by stationary operand (fewer LDW stalls)
                    def sweep(lhs, rhs_const, pss, start, stop):
                        for c in range(nchunk):
                            for j in range(CHUNK // MM_N):
                                m0 = j * MM_N
                                sl = slice(c * CHUNK + m0, c * CHUNK + m0 + MM_N)
                                nc.tensor.matmul(
                                    pss[c][:, m0:m0 + MM_N], lhsT=lhs,
                                    rhs=rhs_const[:, sl], start=start, stop=stop,
                                    skip_group_check=True,
                                )
                    sweep(d_hi[:, j0:j0 + nt], MAh, psAs, True, False)
                    sweep(d_lo[:, j0:j0 + nt], MAh, psAs, False, False)
                    sweep(d_hi[:, j0:j0 + nt], MAl, psAs, False, True)
                    sweep(g_hi[:, j0:j0 + nt], MBh, psBs, True, True)
                    for c in range(nchunk):
                        f0 = c * CHUNK
                        e = p_e.tile([nt, CHUNK], BF16)
                        nc.scalar.activation(e[:], psAs[c][:], Exp)
                        t4b = p_t4b.tile([nt, CHUNK], BF16)
                        nc.scalar.copy(t4b[:], psBs[c][:])
                        t2 = p_t2.tile([nt, CHUNK], BF16)
                        nc.vector.tensor_mul(t2[:], u_in[:, f0:f0 + CHUNK], e[:])
                        nc.vector.tensor_add(ut[:, f0:f0 + CHUNK], t2[:], t4b[:])
                        yt = p_yt.tile([nt, CHUNK], BF16)
                        nc.vector.tensor_mul(
                            yt[:], ut[:, f0:f0 + CHUNK], Crep[:nt, f0:f0 + CHUNK]
                        )
                        h0 = f0 // STATE
                        nh = CHUNK // STATE
                        nc.vector.tensor_reduce(
                            y[:, h0:h0 + nh],
                            yt[:].rearrange("p (h s) -> p h s", s=STATE),
                            axis=mybir.AxisListType.X,
                            op=mybir.AluOpType.add,
                        )
                    nc.scalar.dma_start(out=ut_d[r0:r0 + nt, :], in_=ut[:])
                    nc.scalar.dma_start(out=y_d[r0:r0 + nt, :], in_=y[:])

            # software pipeline: delta phase of group i+1 is emitted before the
            # streaming phase of group i
            groups = _groups()
            pend = []
            for gi, (g0, gr, tiles) in enumerate(groups):
                pend.append((g0, gr, tiles, phase_a(g0, gr, tiles)))
                if gi >= 1:
                    b = pend.pop(0)
                    phase_b(b[0], b[1], b[2], b[3])
            for b in pend:
                phase_b(b[0], b[1], b[2], b[3])

    nc.compile()
    return nc


_NC = None


def _get_nc():
    global _NC
    if _NC is None:
        _NC = build()
    return _NC


def _host_consts(W_delta, b_delta, log_A, B, C):
    import ml_dtypes

    A = -np.exp(log_A.astype(np.float64)).astype(np.float32)  # (H, S)
    idx = np.arange(HID)
    MA = np.zeros((HID, F), dtype=np.float32)
    MA[idx[:, None], idx[:, None] * STATE + np.arange(STATE)[None, :]] = A
    MB = np.zeros((HID, F), dtype=np.float32)
    MB[idx[:, None], idx[:, None] * STATE + np.arange(STATE)[None, :]] = B.astype(
        np.float32
    )
    MAh = MA.astype(ml_dtypes.bfloat16)
    MAl = (MA - MAh.astype(np.float32)).astype(ml_dtypes.bfloat16)
    MBh = MB.astype(ml_dtypes.bfloat16)
    Crep = np.broadcast_to(
        C.astype(np.float32).reshape(1, F), (HID, F)
    ).astype(ml_dtypes.bfloat16)
    return {
        "WdT": np.ascontiguousarray(W_delta.T.astype(np.float32)),
        "bD": b_delta.astype(np.float32).reshape(HID, 1),
        "MAh": MAh,
        "MAl": MAl,
        "MBh": MBh,
        "Crep": Crep,
        "ident": np.eye(HID, dtype=np.float32),
    }


def make_in_maps(inputs):
    import ml_dtypes

    h_t = np.asarray(inputs["h_t"], dtype=np.float32)
    u_prev = np.asarray(inputs["u_prev"], dtype=np.float32)
    consts = _host_consts(
        np.asarray(inputs["W_delta"]), np.asarray(inputs["b_delta"]),
        np.asarray(inputs["log_A"]), np.asarray(inputs["B"]),
        np.asarray(inputs["C"]),
    )
    u_flat = np.ascontiguousarray(u_prev.reshape(N, F)).astype(ml_dtypes.bfloat16)
    in_maps = []
    for i in range(NCORES):
        r0 = i * R
        m = {
            "h_t": np.ascontiguousarray(h_t[r0:r0 + R]),
            "u_prev": u_flat[r0:r0 + R],
        }
        m.update(consts)
        in_maps.append(m)
    return in_maps


def gather_outputs(results):
    y = np.concatenate([results[i]["y_t"] for i in range(NCORES)], axis=0)
    ut = np.concatenate(
        [results[i]["u_t"].astype(np.float32) for i in range(NCORES)], axis=0
    )
    return y, ut.reshape(N, HID, STATE)


def kernel(h_t, u_prev, W_delta, b_delta, log_A, B, C):
    inputs = dict(h_t=h_t, u_prev=u_prev, W_delta=W_delta, b_delta=b_delta,
                  log_A=log_A, B=B, C=C)
    in_maps = make_in_maps(inputs)
    nc = _get_nc()
    res = run_bass_kernel_spmd(nc, in_maps, list(range(NCORES)))
    return gather_outputs(res.results)


# revision 14
# speedup vs baseline: 1.8175x; 1.0566x over previous
"""AdaptiveSSM forward on 8 Trainium2 NeuronCores (data-parallel over N).

y_t, u_t = SSM(h_t, u_prev, W_delta, b_delta, log_A, B, C)
  delta = softplus(h @ W^T + b)                  (N, H)
  u_t   = u_prev * exp(delta[:,:,None]*A) + delta[:,:,None]*h[:,:,None]*B
  y_t   = sum_s(u_t * C)

Per-core layout: rows on SBUF partitions (tiles of 128 rows), (H,S)=2048 on
the free dim (contiguous per row -> full-rate DMA).  The broadcasts
delta (x) A and (delta*h) (x) B are produced by the TensorEngine as matmuls
against host-precomputed block-diagonal constants MA/MB
(MA[k, h*16+s] = A[h,s] if k==h else 0) with the transposed delta as the
stationary operand, so they land in PSUM without any DVE broadcast pass.
delta itself is computed transposed ([H, n]) so the Linear bias and the
softplus run on the Scalar engine with a per-partition bias.
"""

import sys
from contextlib import ExitStack

sys.path.insert(0, "/opt/trn_rl_repo")

import numpy as np

import concourse.bass as bass
import concourse.tile as tile
from concourse import bacc, mybir
from concourse.bass_utils import run_bass_kernel_spmd

N = 50000
HID = 128
STATE = 16
F = HID * STATE  # 2048
NCORES = 8
R = N // NCORES  # 6250 rows per core
TILE_ROWS = 128
CHUNK = 1024  # free-dim chunk for the elementwise pipeline (2 PSUM banks)
MM_N = 512  # max fp32 moving free dim per matmul
NEWTON_ITERS = 3
GROUP_ROWS = 512  # rows per delta-phase group (4 row-tiles)

F32 = mybir.dt.float32
BF16 = mybir.dt.bfloat16


def _row_tiles():
    tiles = []
    r0 = 0
    while r0 < R:
        nt = min(TILE_ROWS, R - r0)
        tiles.append((r0, nt))
        r0 += nt
    return tiles


def _groups():
    """Partition R rows into groups of row-tiles: [(g0, [(r0, nt), ...]), ...]"""
    out = []
    g0 = 0
    while g0 < R:
        gr = min(GROUP_ROWS, R - g0)
        tiles = []
        r0 = g0
        while r0 < g0 + gr:
            nt = min(TILE_ROWS, g0 + gr - r0)
            tiles.append((r0, nt))
            r0 += nt
        out.append((g0, gr, tiles))
        g0 += gr
    return out


def build():
    nc = bacc.Bacc()

    h_d = nc.declare_dram_parameter("h_t", [R, HID], F32, isOutput=False)
    u_d = nc.declare_dram_parameter("u_prev", [R, F], BF16, isOutput=False)
    wdT_d = nc.declare_dram_parameter("WdT", [HID, HID], F32, isOutput=False)
    bD_d = nc.declare_dram_parameter("bD", [HID, 1], F32, isOutput=False)
    MAh_d = nc.declare_dram_parameter("MAh", [HID, F], BF16, isOutput=False)
    MBh_d = nc.declare_dram_parameter("MBh", [HID, F], BF16, isOutput=False)
    Crep_d = nc.declare_dram_parameter("Crep", [HID, F], BF16, isOutput=False)
    ident_d = nc.declare_dram_parameter("ident", [HID, HID], F32, isOutput=False)
    y_d = nc.declare_dram_parameter("y_t", [R, HID], F32, isOutput=True)
    ut_d = nc.declare_dram_parameter("u_t", [R, F], BF16, isOutput=True)

    Exp = mybir.ActivationFunctionType.Exp
    Relu = mybir.ActivationFunctionType.Relu

    with tile.TileContext(nc) as tc:
        with ExitStack() as ctx:
            ep = ctx.enter_context
            consts = ep(tc.tile_pool(name="consts", bufs=1))
            p_u = ep(tc.tile_pool(name="u_in", bufs=3))
            p_h = ep(tc.tile_pool(name="h_in", bufs=3))
            p_hTg = ep(tc.tile_pool(name="hTg", bufs=2))
            p_z0 = ep(tc.tile_pool(name="z0", bufs=2))
            p_z = ep(tc.tile_pool(name="z", bufs=2))
            p_q0 = ep(tc.tile_pool(name="q0", bufs=2))
            p_w = ep(tc.tile_pool(name="w", bufs=2))
            p_p1 = ep(tc.tile_pool(name="p1", bufs=2))
            p_p2 = ep(tc.tile_pool(name="p2", bufs=2))
            p_p3 = ep(tc.tile_pool(name="p3", bufs=2))
            p_a = ep(tc.tile_pool(name="a", bufs=2))
            p_dT = ep(tc.tile_pool(name="dT", bufs=2))
            p_gT = ep(tc.tile_pool(name="gT", bufs=2))
            p_dhi = ep(tc.tile_pool(name="dhi", bufs=2))
            p_dlo = ep(tc.tile_pool(name="dlo", bufs=2))
            p_ghi = ep(tc.tile_pool(name="ghi", bufs=2))
            p_e = ep(tc.tile_pool(name="e", bufs=3))
            p_t2 = ep(tc.tile_pool(name="t2", bufs=3))
            p_yt = ep(tc.tile_pool(name="yt", bufs=3))
            p_t4b = ep(tc.tile_pool(name="t4b", bufs=3))
            p_ut = ep(tc.tile_pool(name="ut", bufs=3))
            p_y = ep(tc.tile_pool(name="y", bufs=3))
            p_psT = ep(tc.tile_pool(name="psT", bufs=1, space="PSUM"))
            p_psD = ep(tc.tile_pool(name="psD", bufs=1, space="PSUM"))
            p_psA = ep(tc.tile_pool(name="psA", bufs=2, space="PSUM"))
            p_psB = ep(tc.tile_pool(name="psB", bufs=1, space="PSUM"))

            wdT = consts.tile([HID, HID], F32)
            nc.sync.dma_start(out=wdT[:], in_=wdT_d[:, :])
            bD = consts.tile([HID, 1], F32)
            nc.sync.dma_start(out=bD[:], in_=bD_d[:, :])
            MAh = consts.tile([HID, F], BF16)
            nc.sync.dma_start(out=MAh[:], in_=MAh_d[:, :])
            MBh = consts.tile([HID, F], BF16)
            nc.sync.dma_start(out=MBh[:], in_=MBh_d[:, :])
            Crep = consts.tile([HID, F], BF16)
            nc.sync.dma_start(out=Crep[:], in_=Crep_d[:, :])
            ident = consts.tile([HID, HID], F32)
            nc.sync.dma_start(out=ident[:], in_=ident_d[:, :])

            def phase_a(g0, gr, tiles):
                """delta/softplus for one group -> (d_hi, d_lo, g_hi) bf16."""
                hTg = p_hTg.tile([HID, gr], F32)
                for r0, nt in tiles:
                    h_in = p_h.tile([nt, HID], F32)
                    nc.sync.dma_start(out=h_in[:], in_=h_d[r0:r0 + nt, :])
                    psT = p_psT.tile([HID, nt], F32)
                    nc.tensor.transpose(psT[:], h_in[:], ident[:nt, :nt])
                    nc.scalar.copy(hTg[:, r0 - g0:r0 - g0 + nt], psT[:])
                psD = p_psD.tile([HID, gr], F32)
                nc.tensor.matmul(psD[:], lhsT=wdT[:], rhs=hTg[:], start=True, stop=True)
                # softplus(pre+b) via Newton: d_K = relu + p1+..+pK - K,
                # p1 = (1+e^x)e^{-relu(x)}, p_{k+1} = p_k * exp(1 - p_k)
                z0 = p_z0.tile([HID, gr], F32)
                nc.scalar.activation(z0[:], psD[:], Exp, bias=bD[:])
                z = p_z.tile([HID, gr], F32)
                nc.vector.tensor_scalar_add(z[:], z0[:], 1.0)
                q0 = p_q0.tile([HID, gr], F32)
                nc.scalar.activation(q0[:], psD[:], Relu, bias=bD[:])
                w = p_w.tile([HID, gr], F32)
                nc.scalar.activation(w[:], q0[:], Exp, scale=-1.0)
                p1 = p_p1.tile([HID, gr], F32)
                nc.gpsimd.tensor_tensor(p1[:], z[:], w[:], mybir.AluOpType.mult)
                w2 = p_w.tile([HID, gr], F32)
                nc.scalar.activation(w2[:], p1[:], Exp, scale=-1.0, bias=1.0)
                p2 = p_p2.tile([HID, gr], F32)
                nc.gpsimd.tensor_tensor(p2[:], p1[:], w2[:], mybir.AluOpType.mult)
                w3 = p_w.tile([HID, gr], F32)
                nc.scalar.activation(w3[:], p2[:], Exp, scale=-1.0, bias=1.0)
                p3 = p_p3.tile([HID, gr], F32)
                nc.gpsimd.tensor_tensor(p3[:], p2[:], w3[:], mybir.AluOpType.mult)
                a1 = p_a.tile([HID, gr], F32)
                nc.vector.tensor_add(a1[:], p1[:], p2[:])
                a2 = p_a.tile([HID, gr], F32)
                nc.vector.tensor_add(a2[:], p3[:], q0[:])
                dT = p_dT.tile([HID, gr], F32)
                nc.vector.scalar_tensor_tensor(
                    dT[:], a1[:], float(-NEWTON_ITERS), a2[:],
                    mybir.AluOpType.add, mybir.AluOpType.add,
                )
                gT = p_gT.tile([HID, gr], F32)
                nc.vector.tensor_mul(gT[:], dT[:], hTg[:])
                d_hi = p_dhi.tile([HID, gr], BF16)
                nc.scalar.copy(d_hi[:], dT[:])
                d_lo = p_dlo.tile([HID, gr], BF16)
                nc.gpsimd.tensor_tensor(
                    d_lo[:], dT[:], d_hi[:], mybir.AluOpType.subtract
                )
                g_hi = p_ghi.tile([HID, gr], BF16)
                nc.scalar.copy(g_hi[:], gT[:])
                return d_hi, d_lo, g_hi

            def phase_b(g0, gr, tiles, dhdlgh):
                d_hi, d_lo, g_hi = dhdlgh
                for r0, nt in tiles:
                    j0 = r0 - g0
                    u_in = p_u.tile([nt, F], BF16)
                    nc.sync.dma_start(out=u_in[:], in_=u_d[r0:r0 + nt, :])
                    ut = p_ut.tile([nt, F], BF16)
                    y = p_y.tile([nt, HID], F32)
                    psAs = []
                    psBs = []
                    nchunk = F // CHUNK
                    for c in range(nchunk):
                        psAs.append(p_psA.tile([nt, CHUNK], F32, name="psA", tag="psA"))
                        psBs.append(p_psB.tile([nt, CHUNK], F32, name="psB", tag="psB"))
                    # matmuls grouped by stationary operand (fewer LDW stalls)
                    def sweep(lhs, rhs_const, pss, start, stop):
                        for c in range(nchunk):
                            for j in range(CHUNK // MM_N):
                                m0 = j * MM_N
                                sl = slice(c * CHUNK + m0, c * CHUNK + m0 + MM_N)
                                nc.tensor.matmul(
                                    pss[c][:, m0:m0 + MM_N], lhsT=lhs,
                                    rhs=rhs_const[:, sl], start=start, stop=stop,
                                    skip_group_check=True,
                                )
                    sweep(d_hi[:, j0:j0 + nt], MAh, psAs, True, False)
                    sweep(d_lo[:, j0:j0 + nt], MAh, psAs, False, True)
                    sweep(g_hi[:, j0:j0 + nt], MBh, psBs, True, True)
                    for c in range(nchunk):
                        f0 = c * CHUNK
                        e = p_e.tile([nt, CHUNK], BF16)
                        nc.scalar.activation(e[:], psAs[c][:], Exp)
                        t4b = p_t4b.tile([nt, CHUNK], BF16)
                        nc.scalar.copy(t4b[:], psBs[c][:])
                        t2 = p_t2.tile([nt, CHUNK], BF16)
                        nc.vector.tensor_mul(t2[:], u_in[:, f0:f0 + CHUNK], e[:])
                        nc.vector.tensor_add(ut[:, f0:f0 + CHUNK], t2[:], t4b[:])
                        yt = p_yt.tile([nt, CHUNK], BF16)
                        nc.vector.tensor_mul(
                            yt[:], ut[:, f0:f0 + CHUNK], Crep[:nt, f0:f0 + CHUNK]
                        )
                        h0 = f0 // STATE
                        nh = CHUNK // STATE
                        nc.vector.tensor_reduce(
                            y[:, h0:h0 + nh],
                            yt[:].rearrange("p (h s) -> p h s", s=STATE),
                            axis=mybir.AxisListType.X,
                            op=mybir.AluOpType.add,
                        )
                    nc.sync.dma_start(out=ut_d[r0:r0 + nt, :], in_=ut[:])
                    nc.sync.dma_start(out=y_d[r0:r0 + nt, :], in_=y[:])

            # software pipeline: delta phase of group i+1 is emitted before the
            # streaming phase of group i
            groups = _groups()
            pend = []
            for gi, (g0, gr, tiles) in enumerate(groups):
                pend.append((g0, gr, tiles, phase_a(g0, gr, tiles)))
                if gi >= 1:
                    b = pend.pop(0)
                    phase_b(b[0], b[1], b[2], b[3])
            for b in pend:
                phase_b(b[0], b[1], b[2], b[3])

    nc.compile()
    return nc


_NC = None


def _get_nc():
    global _NC
    if _NC is None:
        _NC = build()
    return _NC


def _host_consts(W_delta, b_delta, log_A, B, C):
    import ml_dtypes

    A = -np.exp(log_A.astype(np.float64)).astype(np.float32)  # (H, S)
    idx = np.arange(HID)
    MA = np.zeros((HID, F), dtype=np.float32)
    MA[idx[:, None], idx[:, None] * STATE + np.arange(STATE)[None, :]] = A
    MB = np.zeros((HID, F), dtype=np.float32)
    MB[idx[:, None], idx[:, None] * STATE + np.arange(STATE)[None, :]] = B.astype(
        np.float32
    )
    MAh = MA.astype(ml_dtypes.bfloat16)
    MBh = MB.astype(ml_dtypes.bfloat16)
    Crep = np.broadcast_to(
        C.astype(np.float32).reshape(1, F), (HID, F)
    ).astype(ml_dtypes.bfloat16)
    return {
        "WdT": np.ascontiguousarray(W_delta.T.astype(np.float32)),
        "bD": b_delta.astype(np.float32).reshape(HID, 1),
        "MAh": MAh,
        "MBh": MBh,
        "Crep": Crep,
        "ident": np.eye(HID, dtype=np.float32),
    }


def make_in_maps(inputs):
    import ml_dtypes

    h_t = np.asarray(inputs["h_t"], dtype=np.float32)
    u_prev = np.asarray(inputs["u_prev"], dtype=np.float32)
    consts = _host_consts(
        np.asarray(inputs["W_delta"]), np.asarray(inputs["b_delta"]),
        np.asarray(inputs["log_A"]), np.asarray(inputs["B"]),
        np.asarray(inputs["C"]),
    )
    u_flat = np.ascontiguousarray(u_prev.reshape(N, F)).astype(ml_dtypes.bfloat16)
    in_maps = []
    for i in range(NCORES):
        r0 = i * R
        m = {
            "h_t": np.ascontiguousarray(h_t[r0:r0 + R]),
            "u_prev": u_flat[r0:r0 + R],
        }
        m.update(consts)
        in_maps.append(m)
    return in_maps


def gather_outputs(results):
    y = np.concatenate([results[i]["y_t"] for i in range(NCORES)], axis=0)
    ut = np.concatenate(
        [results[i]["u_t"].astype(np.float32) for i in range(NCORES)], axis=0
    )
    return y, ut.reshape(N, HID, STATE)


def kernel(h_t, u_prev, W_delta, b_delta, log_A, B, C):
    inputs = dict(h_t=h_t, u_prev=u_prev, W_delta=W_delta, b_delta=b_delta,
                  log_A=log_A, B=B, C=C)
    in_maps = make_in_maps(inputs)
    nc = _get_nc()
    res = run_bass_kernel_spmd(nc, in_maps, list(range(NCORES)))
    return gather_outputs(res.results)


# revision 15
# speedup vs baseline: 1.8598x; 1.0232x over previous
"""AdaptiveSSM forward on 8 Trainium2 NeuronCores (data-parallel over N).

y_t, u_t = SSM(h_t, u_prev, W_delta, b_delta, log_A, B, C)
  delta = softplus(h @ W^T + b)                  (N, H)
  u_t   = u_prev * exp(delta[:,:,None]*A) + delta[:,:,None]*h[:,:,None]*B
  y_t   = sum_s(u_t * C)

Per-core layout: rows on SBUF partitions (tiles of 128 rows), (H,S)=2048 on
the free dim (contiguous per row -> full-rate DMA).  The broadcasts
delta (x) A and (delta*h) (x) B are produced by the TensorEngine as matmuls
against host-precomputed block-diagonal constants MA/MB
(MA[k, h*16+s] = A[h,s] if k==h else 0) with the transposed delta as the
stationary operand, so they land in PSUM without any DVE broadcast pass.
delta itself is computed transposed ([H, n]) so the Linear bias and the
softplus run on the Scalar engine with a per-partition bias.
"""

import sys
from contextlib import ExitStack

sys.path.insert(0, "/opt/trn_rl_repo")

import numpy as np

import concourse.bass as bass
import concourse.tile as tile
from concourse import bacc, mybir
from concourse.bass_utils import run_bass_kernel_spmd

N = 50000
HID = 128
STATE = 16
F = HID * STATE  # 2048
NCORES = 8
R = N // NCORES  # 6250 rows per core
TILE_ROWS = 128
CHUNK = 1024  # free-dim chunk for the elementwise pipeline (2 PSUM banks)
MM_N = 512  # max fp32 moving free dim per matmul
NEWTON_ITERS = 3
GROUP_ROWS = 512  # rows per delta-phase group (4 row-tiles)

F32 = mybir.dt.float32
BF16 = mybir.dt.bfloat16


def _row_tiles():
    tiles = []
    r0 = 0
    while r0 < R:
        nt = min(TILE_ROWS, R - r0)
        tiles.append((r0, nt))
        r0 += nt
    return tiles


def _groups():
    """Partition R rows into groups of row-tiles: [(g0, [(r0, nt), ...]), ...]"""
    out = []
    g0 = 0
    while g0 < R:
        gr = min(GROUP_ROWS, R - g0)
        tiles = []
        r0 = g0
        while r0 < g0 + gr:
            nt = min(TILE_ROWS, g0 + gr - r0)
            tiles.append((r0, nt))
            r0 += nt
        out.append((g0, gr, tiles))
        g0 += gr
    return out


def build():
    nc = bacc.Bacc()

    h_d = nc.declare_dram_parameter("h_t", [R, HID], F32, isOutput=False)
    u_d = nc.declare_dram_parameter("u_prev", [R, F], BF16, isOutput=False)
    wdT_d = nc.declare_dram_parameter("WdT", [HID, HID], F32, isOutput=False)
    bD_d = nc.declare_dram_parameter("bD", [HID, 1], F32, isOutput=False)
    MAh_d = nc.declare_dram_parameter("MAh", [HID, F], BF16, isOutput=False)
    MBh_d = nc.declare_dram_parameter("MBh", [HID, F], BF16, isOutput=False)
    Crep_d = nc.declare_dram_parameter("Crep", [HID, F], BF16, isOutput=False)
    ident_d = nc.declare_dram_parameter("ident", [HID, HID], F32, isOutput=False)
    y_d = nc.declare_dram_parameter("y_t", [R, HID], F32, isOutput=True)
    ut_d = nc.declare_dram_parameter("u_t", [R, F], BF16, isOutput=True)

    Exp = mybir.ActivationFunctionType.Exp
    Relu = mybir.ActivationFunctionType.Relu

    with tile.TileContext(nc) as tc:
        with ExitStack() as ctx:
            ep = ctx.enter_context
            consts = ep(tc.tile_pool(name="consts", bufs=1))
            p_u = ep(tc.tile_pool(name="u_in", bufs=3))
            p_h = ep(tc.tile_pool(name="h_in", bufs=3))
            p_hTg = ep(tc.tile_pool(name="hTg", bufs=3))
            p_z0 = ep(tc.tile_pool(name="z0", bufs=2))
            p_z = ep(tc.tile_pool(name="z", bufs=2))
            p_q0 = ep(tc.tile_pool(name="q0", bufs=2))
            p_w = ep(tc.tile_pool(name="w", bufs=2))
            p_p1 = ep(tc.tile_pool(name="p1", bufs=2))
            p_p2 = ep(tc.tile_pool(name="p2", bufs=2))
            p_p3 = ep(tc.tile_pool(name="p3", bufs=2))
            p_a = ep(tc.tile_pool(name="a", bufs=2))
            p_dT = ep(tc.tile_pool(name="dT", bufs=2))
            p_gT = ep(tc.tile_pool(name="gT", bufs=2))
            p_dhi = ep(tc.tile_pool(name="dhi", bufs=3))
            p_ghi = ep(tc.tile_pool(name="ghi", bufs=3))
            p_e = ep(tc.tile_pool(name="e", bufs=3))
            p_t2 = ep(tc.tile_pool(name="t2", bufs=3))
            p_yt = ep(tc.tile_pool(name="yt", bufs=3))
            p_t4b = ep(tc.tile_pool(name="t4b", bufs=3))
            p_ut = ep(tc.tile_pool(name="ut", bufs=3))
            p_y = ep(tc.tile_pool(name="y", bufs=3))
            p_psT = ep(tc.tile_pool(name="psT", bufs=1, space="PSUM"))
            p_psD = ep(tc.tile_pool(name="psD", bufs=1, space="PSUM"))
            p_psA = ep(tc.tile_pool(name="psA", bufs=2, space="PSUM"))
            p_psB = ep(tc.tile_pool(name="psB", bufs=1, space="PSUM"))

            wdT = consts.tile([HID, HID], F32)
            nc.sync.dma_start(out=wdT[:], in_=wdT_d[:, :])
            bD = consts.tile([HID, 1], F32)
            nc.sync.dma_start(out=bD[:], in_=bD_d[:, :])
            MAh = consts.tile([HID, F], BF16)
            nc.sync.dma_start(out=MAh[:], in_=MAh_d[:, :])
            MBh = consts.tile([HID, F], BF16)
            nc.sync.dma_start(out=MBh[:], in_=MBh_d[:, :])
            Crep = consts.tile([HID, F], BF16)
            nc.sync.dma_start(out=Crep[:], in_=Crep_d[:, :])
            ident = consts.tile([HID, HID], F32)
            nc.sync.dma_start(out=ident[:], in_=ident_d[:, :])

            def phase_a(g0, gr, tiles):
                """delta/softplus for one group -> (d_hi, d_lo, g_hi) bf16."""
                hTg = p_hTg.tile([HID, gr], F32)
                for r0, nt in tiles:
                    h_in = p_h.tile([nt, HID], F32)
                    nc.sync.dma_start(out=h_in[:], in_=h_d[r0:r0 + nt, :])
                    psT = p_psT.tile([HID, nt], F32)
                    nc.tensor.transpose(psT[:], h_in[:], ident[:nt, :nt])
                    nc.scalar.copy(hTg[:, r0 - g0:r0 - g0 + nt], psT[:])
                psD = p_psD.tile([HID, gr], F32)
                nc.tensor.matmul(psD[:], lhsT=wdT[:], rhs=hTg[:], start=True, stop=True)
                # softplus(pre+b) via Newton: d_K = relu + p1+..+pK - K,
                # p1 = (1+e^x)e^{-relu(x)}, p_{k+1} = p_k * exp(1 - p_k)
                z0 = p_z0.tile([HID, gr], F32)
                nc.scalar.activation(z0[:], psD[:], Exp, bias=bD[:])
                z = p_z.tile([HID, gr], F32)
                nc.vector.tensor_scalar_add(z[:], z0[:], 1.0)
                q0 = p_q0.tile([HID, gr], F32)
                nc.scalar.activation(q0[:], psD[:], Relu, bias=bD[:])
                w = p_w.tile([HID, gr], F32)
                nc.scalar.activation(w[:], q0[:], Exp, scale=-1.0)
                p1 = p_p1.tile([HID, gr], F32)
                nc.gpsimd.tensor_tensor(p1[:], z[:], w[:], mybir.AluOpType.mult)
                w2 = p_w.tile([HID, gr], F32)
                nc.scalar.activation(w2[:], p1[:], Exp, scale=-1.0, bias=1.0)
                p2 = p_p2.tile([HID, gr], F32)
                nc.gpsimd.tensor_tensor(p2[:], p1[:], w2[:], mybir.AluOpType.mult)
                w3 = p_w.tile([HID, gr], F32)
                nc.scalar.activation(w3[:], p2[:], Exp, scale=-1.0, bias=1.0)
                p3 = p_p3.tile([HID, gr], F32)
                nc.gpsimd.tensor_tensor(p3[:], p2[:], w3[:], mybir.AluOpType.mult)
                a1 = p_a.tile([HID, gr], F32)
                nc.vector.tensor_add(a1[:], p1[:], p2[:])
                a2 = p_a.tile([HID, gr], F32)
                nc.vector.tensor_add(a2[:], p3[:], q0[:])
                dT = p_dT.tile([HID, gr], F32)
                nc.vector.scalar_tensor_tensor(
                    dT[:], a1[:], float(-NEWTON_ITERS), a2[:],
                    mybir.AluOpType.add, mybir.AluOpType.add,
                )
                gT = p_gT.tile([HID, gr], F32)
                nc.vector.tensor_mul(gT[:], dT[:], hTg[:])
                d_hi = p_dhi.tile([HID, gr], BF16)
                nc.scalar.copy(d_hi[:], dT[:])
                g_hi = p_ghi.tile([HID, gr], BF16)
                nc.scalar.copy(g_hi[:], gT[:])
                return d_hi, g_hi

            def phase_b(g0, gr, tiles, dhdlgh):
                d_hi, g_hi = dhdlgh
                for r0, nt in tiles:
                    j0 = r0 - g0
                    u_in = p_u.tile([nt, F], BF16)
                    nc.sync.dma_start(out=u_in[:], in_=u_d[r0:r0 + nt, :])
                    ut = p_ut.tile([nt, F], BF16)
                    y = p_y.tile([nt, HID], F32)
                    psAs = []
                    psBs = []
                    nchunk = F // CHUNK
                    for c in range(nchunk):
                        psAs.append(p_psA.tile([nt, CHUNK], F32, name="psA", tag="psA"))
                        psBs.append(p_psB.tile([nt, CHUNK], F32, name="psB", tag="psB"))
                    # matmuls grouped by stationary operand (fewer LDW stalls)
                    def sweep(lhs, rhs_const, pss, start, stop):
                        for c in range(nchunk):
                            for j in range(CHUNK // MM_N):
                                m0 = j * MM_N
                                sl = slice(c * CHUNK + m0, c * CHUNK + m0 + MM_N)
                                nc.tensor.matmul(
                                    pss[c][:, m0:m0 + MM_N], lhsT=lhs,
                                    rhs=rhs_const[:, sl], start=start, stop=stop,
                                    skip_group_check=True,
                                )
                    sweep(d_hi[:, j0:j0 + nt], MAh, psAs, True, True)
                    sweep(g_hi[:, j0:j0 + nt], MBh, psBs, True, True)
                    for c in range(nchunk):
                        f0 = c * CHUNK
                        e = p_e.tile([nt, CHUNK], BF16)
                        nc.scalar.activation(e[:], psAs[c][:], Exp)
                        t4b = p_t4b.tile([nt, CHUNK], BF16)
                        nc.scalar.copy(t4b[:], psBs[c][:])
                        t2 = p_t2.tile([nt, CHUNK], BF16)
                        nc.vector.tensor_mul(t2[:], u_in[:, f0:f0 + CHUNK], e[:])
                        nc.vector.tensor_add(ut[:, f0:f0 + CHUNK], t2[:], t4b[:])
                        yt = p_yt.tile([nt, CHUNK], BF16)
                        nc.vector.tensor_mul(
                            yt[:], ut[:, f0:f0 + CHUNK], Crep[:nt, f0:f0 + CHUNK]
                        )
                        h0 = f0 // STATE
                        nh = CHUNK // STATE
                        nc.vector.tensor_reduce(
                            y[:, h0:h0 + nh],
                            yt[:].rearrange("p (h s) -> p h s", s=STATE),
                            axis=mybir.AxisListType.X,
                            op=mybir.AluOpType.add,
                        )
                    nc.sync.dma_start(out=ut_d[r0:r0 + nt, :], in_=ut[:])
                    nc.sync.dma_start(out=y_d[r0:r0 + nt, :], in_=y[:])

            # software pipeline: delta phase of group i+1 is emitted before the
            # streaming phase of group i
            groups = _groups()
            pend = []
            for gi, (g0, gr, tiles) in enumerate(groups):
                pend.append((g0, gr, tiles, phase_a(g0, gr, tiles)))
                if gi >= 2:
                    b = pend.pop(0)
                    phase_b(b[0], b[1], b[2], b[3])
            for b in pend:
                phase_b(b[0], b[1], b[2], b[3])

    nc.compile()
    return nc


_NC = None


def _get_nc():
    global _NC
    if _NC is None:
        _NC = build()
    return _NC


def _host_consts(W_delta, b_delta, log_A, B, C):
    import ml_dtypes

    A = -np.exp(log_A.astype(np.float64)).astype(np.float32)  # (H, S)
    idx = np.arange(HID)
    MA = np.zeros((HID, F), dtype=np.float32)
    MA[idx[:, None], idx[:, None] * STATE + np.arange(STATE)[None, :]] = A
    MB = np.zeros((HID, F), dtype=np.float32)
    MB[idx[:, None], idx[:, None] * STATE + np.arange(STATE)[None, :]] = B.astype(
        np.float32
    )
    MAh = MA.astype(ml_dtypes.bfloat16)
    MBh = MB.astype(ml_dtypes.bfloat16)
    Crep = np.broadcast_to(
        C.astype(np.float32).reshape(1, F), (HID, F)
    ).astype(ml_dtypes.bfloat16)
    return {
        "WdT": np.ascontiguousarray(W_delta.T.astype(np.float32)),
        "bD": b_delta.astype(np.float32).reshape(HID, 1),
        "MAh": MAh,
        "MBh": MBh,
        "Crep": Crep,
        "ident": np.eye(HID, dtype=np.float32),
    }


def make_in_maps(inputs):
    import ml_dtypes

    h_t = np.asarray(inputs["h_t"], dtype=np.float32)
    u_prev = np.asarray(inputs["u_prev"], dtype=np.float32)
    consts = _host_consts(
        np.asarray(inputs["W_delta"]), np.asarray(inputs["b_delta"]),
        np.asarray(inputs["log_A"]), np.asarray(inputs["B"]),
        np.asarray(inputs["C"]),
    )
    u_flat = np.ascontiguousarray(u_prev.reshape(N, F)).astype(ml_dtypes.bfloat16)
    in_maps = []
    for i in range(NCORES):
        r0 = i * R
        m = {
            "h_t": np.ascontiguousarray(h_t[r0:r0 + R]),
            "u_prev": u_flat[r0:r0 + R],
        }
        m.update(consts)
        in_maps.append(m)
    return in_maps


def gather_outputs(results):
    y = np.concatenate([results[i]["y_t"] for i in range(NCORES)], axis=0)
    ut = np.concatenate(
        [results[i]["u_t"].astype(np.float32) for i in range(NCORES)], axis=0
    )
    return y, ut.reshape(N, HID, STATE)


def kernel(h_t, u_prev, W_delta, b_delta, log_A, B, C):
    inputs = dict(h_t=h_t, u_prev=u_prev, W_delta=W_delta, b_delta=b_delta,
                  log_A=log_A, B=B, C=C)
    in_maps = make_in_maps(inputs)
    nc = _get_nc()
    res = run_bass_kernel_spmd(nc, in_maps, list(range(NCORES)))
    return gather_outputs(res.results)
